# revision 13
# baseline (speedup 1.0000x reference)
"""BERT encoder layer (B=2, S=2048, H=768, NH=12, F=3072) on 8 TRN2 NeuronCores.

Sharding: pure data-parallel over (batch, query-chunk): core c handles batch
c//4, query rows (c%4)*512 .. +512.  Each core redundantly projects K/V for the
full 2048-token sequence of its batch (no collectives needed).

Numerics: bf16 matmul inputs with f32 PSUM accumulation; softmax without
max-subtraction (scores for this model are in [-3, 3]); normalization by the
softmax denominator is applied to the small ctx^T tile instead of the S^2
attention matrix.  LayerNorm stats in f32.

Layout strategy: "transposed activations" — projections produce K^T/Q^T
(feature-major) and V (token-major) so that attention scores^T, exp, ctx^T and
all later matmuls need no transposes except one 768x512 transpose of an1.
Attention head pairs are packed into the 128-wide PE array via tile_position
(Dh=64).
"""

import sys

for _p in ("/opt/trn_rl_repo",):
    if _p not in sys.path:
        sys.path.insert(0, _p)

import numpy as np
import ml_dtypes

H = 768
F = 3072
NH = 12
DH = 64
S = 2048
B = 2
QC = 512          # query rows per core
KH = H // 128     # 6 contraction chunks for H
KF = F // 128     # 24 contraction chunks for F
NT = S // 128     # 16 token tiles per sequence
QT = QC // 128    # 4 token tiles per core's query chunk
NPAIR = NH // 2   # 6 head pairs
EPS = 1e-5

BF = ml_dtypes.bfloat16

_CACHE = {}


def _build_module(act="Gelu"):
    """act: activation name for the FFN ("Gelu" for real runs; CoreSim lacks
    Gelu so dev-sim uses "Tanh")."""
    import concourse.bass as bass
    import concourse.tile as tile
    from concourse import bacc, mybir
    from concourse.masks import make_identity
    from contextlib import ExitStack

    bf16 = mybir.dt.bfloat16
    f32 = mybir.dt.float32
    ts = bass.ts
    AF = mybir.ActivationFunctionType

    nc = bacc.Bacc("TRN2", target_bir_lowering=False, debug=False)

    d_xT = nc.dram_tensor("xT", [128, KH, S], bf16, kind="ExternalInput").ap()
    d_xqT = nc.dram_tensor("xqT", [128, KH, QC], bf16, kind="ExternalInput").ap()
    d_xq = nc.dram_tensor("xq", [128, QT, H], f32, kind="ExternalInput").ap()
    d_wq = nc.dram_tensor("wq", [128, KH, H], bf16, kind="ExternalInput").ap()
    d_wk = nc.dram_tensor("wk", [128, KH, H], bf16, kind="ExternalInput").ap()
    d_wv = nc.dram_tensor("wv", [128, KH, H], bf16, kind="ExternalInput").ap()
    d_wa1 = nc.dram_tensor("wa1", [128, KH, H], bf16, kind="ExternalInput").ap()
    d_wa2 = nc.dram_tensor("wa2", [128, KH, H], bf16, kind="ExternalInput").ap()
    d_w1 = nc.dram_tensor("w1", [KF, 128, KH, 128], bf16, kind="ExternalInput").ap()
    d_w2 = nc.dram_tensor("w2", [KF, 128, H], bf16, kind="ExternalInput").ap()
    d_out = nc.dram_tensor("out", [128, QT, H], f32, kind="ExternalOutput").ap()

    with tile.TileContext(nc) as tc, ExitStack() as ctx:
        singles = ctx.enter_context(tc.tile_pool(name="singles", bufs=1))

        # ---- resident SBUF tensors -------------------------------------
        XT = singles.tile([128, KH, S], bf16)
        nc.sync.dma_start(out=XT[:], in_=d_xT[:])
        WQ = singles.tile([128, KH, H], bf16)
        nc.sync.dma_start(out=WQ[:], in_=d_wq[:])
        WK = singles.tile([128, KH, H], bf16)
        nc.sync.dma_start(out=WK[:], in_=d_wk[:])
        WV = singles.tile([128, KH, H], bf16)
        nc.sync.dma_start(out=WV[:], in_=d_wv[:])
        WA1 = singles.tile([128, KH, H], bf16)
        nc.sync.dma_start(out=WA1[:], in_=d_wa1[:])
        WA2 = singles.tile([128, KH, H], bf16)
        nc.sync.dma_start(out=WA2[:], in_=d_wa2[:])
        XQ = singles.tile([128, QT, H], f32)
        nc.sync.dma_start(out=XQ[:], in_=d_xq[:])
        XQT = singles.tile([128, KH, QC], bf16)
        nc.sync.dma_start(out=XQT[:], in_=d_xqT[:])

        ONES = singles.tile([128, 64], bf16)
        nc.vector.memset(ONES[:], 1.0)
        ONES32 = singles.tile([128, 64], f32)
        nc.vector.memset(ONES32[:], 1.0)
        IDN = singles.tile([128, 128], bf16)
        make_identity(nc, IDN[:])
        EPS_T = singles.tile([128, 1], f32)
        nc.vector.memset(EPS_T[:], EPS)

        V_sb = singles.tile([128, NT, H], bf16)      # V token-major
        QT_sb = singles.tile([128, KH, QC], bf16)    # Q^T feature-major
        CTX_sb = singles.tile([128, KH, QC], bf16)   # ctx^T feature-major
        AN1 = singles.tile([128, QT, H], f32)        # an1 token-major f32
        AN1B = singles.tile([128, QT, H], bf16)      # an1 cast
        AN1T = singles.tile([128, KH, QC], bf16)     # an1^T
        FFT_sb = singles.tile([128, KH, QC], bf16)   # ff^T

        # ================ Phase 1: V (all heads) + Q^T ==================
        with tc.tile_pool(name="proj768", bufs=2, space="PSUM") as proj768:
            for m in range(NT):
                ps = proj768.tile([128, H], mybir.dt.float32)
                for k in range(KH):
                    nc.tensor.matmul(
                        ps[:, 0:512], lhsT=XT[:, k, ts(m, 128)], rhs=WV[:, k, 0:512],
                        start=(k == 0), stop=(k == KH - 1))
                for k in range(KH):
                    nc.tensor.matmul(
                        ps[:, 512:768], lhsT=XT[:, k, ts(m, 128)], rhs=WV[:, k, 512:768],
                        start=(k == 0), stop=(k == KH - 1))
                nc.vector.tensor_copy(V_sb[:, m, :], ps[:])
            for mk in range(KH):
                ps = proj768.tile([128, QC], mybir.dt.float32, tag="qt")
                for k in range(KH):
                    nc.tensor.matmul(
                        ps[:], lhsT=WQ[:, k, ts(mk, 128)],
                        rhs=XQT[:, k, :],
                        start=(k == 0), stop=(k == KH - 1))
                nc.scalar.copy(QT_sb[:, mk, :], ps[:])

        # ================ Phase 2: attention by head pairs ==============
        with tc.tile_pool(name="kt", bufs=2) as kt_pool, \
             tc.tile_pool(name="exp", bufs=4) as exp_pool, \
             tc.tile_pool(name="rz", bufs=2) as rz_pool, \
             tc.tile_pool(name="rbs", bufs=2) as rbs_pool, \
             tc.tile_pool(name="ps_kt", bufs=1, space="PSUM") as ps_kt, \
             tc.tile_pool(name="ps_sc", bufs=4, space="PSUM") as ps_sc, \
             tc.tile_pool(name="ps_ctx", bufs=2, space="PSUM") as ps_ctx, \
             tc.tile_pool(name="ps_z", bufs=1, space="PSUM") as ps_z:
            for p in range(NPAIR):
                hA, hB = 2 * p, 2 * p + 1
                KT = kt_pool.tile([128, NT, 128], bf16)
                for n in range(4):
                    ps = ps_kt.tile([128, 512], mybir.dt.float32)
                    for k in range(KH):
                        nc.tensor.matmul(
                            ps[:], lhsT=WK[:, k, ts(p, 128)], rhs=XT[:, k, ts(n, 512)],
                            start=(k == 0), stop=(k == KH - 1))
                    nc.vector.tensor_copy(
                        KT[:, 4 * n:4 * (n + 1), :].rearrange("p a b -> p (a b)"),
                        ps[:])
                ctxp = ps_ctx.tile([128, QC], mybir.dt.float32)
                zp = ps_z.tile([64, QC], mybir.dt.float32)
                for kc in range(NT):
                    sA = ps_sc.tile([128, QC], mybir.dt.float32, tag="s")
                    sB = ps_sc.tile([128, QC], mybir.dt.float32, tag="s")
                    nc.tensor.matmul(
                        sA[:], lhsT=KT[0:64, kc, :], rhs=QT_sb[0:64, p, :],
                        start=True, stop=True, tile_position=(0, 0))
                    nc.tensor.matmul(
                        sB[:], lhsT=KT[64:128, kc, :], rhs=QT_sb[64:128, p, :],
                        start=True, stop=True, tile_position=(64, 0))
                    eA = exp_pool.tile([128, QC], bf16, tag="e")
                    eB = exp_pool.tile([128, QC], bf16, tag="e")
                    nc.scalar.activation(eA[:], sA[:], AF.Exp)
                    nc.scalar.activation(eB[:], sB[:], AF.Exp)
                    nc.tensor.matmul(
                        ctxp[0:64, :], lhsT=V_sb[:, kc, ts(hA, DH)], rhs=eA[:],
                        start=(kc == 0), stop=(kc == NT - 1), tile_position=(0, 0),
                        skip_group_check=True)
                    nc.tensor.matmul(
                        ctxp[64:128, :], lhsT=V_sb[:, kc, ts(hB, DH)], rhs=eB[:],
                        start=(kc == 0), stop=(kc == NT - 1), tile_position=(0, 64),
                        skip_group_check=True)
                    nc.tensor.matmul(
                        zp[0:1, :], lhsT=ONES[:, 0:1], rhs=eA[:],
                        start=(kc == 0), stop=(kc == NT - 1), tile_position=(0, 0),
                        skip_group_check=True)
                    nc.tensor.matmul(
                        zp[32:33, :], lhsT=ONES[:, 0:1], rhs=eB[:],
                        start=(kc == 0), stop=(kc == NT - 1), tile_position=(0, 32),
                        skip_group_check=True)
                rz = rz_pool.tile([33, QC], mybir.dt.float32)
                nc.vector.reciprocal(rz[0:1, :], zp[0:1, :])
                nc.vector.reciprocal(rz[32:33, :], zp[32:33, :])
                rb = ps_sc.tile([128, QC], mybir.dt.float32, tag="s")
                nc.tensor.matmul(
                    rb[0:64, :], lhsT=ONES32[0:1, 0:64], rhs=rz[0:1, :],
                    start=True, stop=True, tile_position=(0, 0),
                    skip_group_check=True)
                nc.tensor.matmul(
                    rb[64:128, :], lhsT=ONES32[32:33, 0:64], rhs=rz[32:33, :],
                    start=True, stop=True, tile_position=(32, 64),
                    skip_group_check=True)
                rbs = rbs_pool.tile([128, QC], mybir.dt.float32)
                nc.vector.tensor_copy(rbs[:], rb[:])
                nc.vector.tensor_mul(CTX_sb[:, p, :], ctxp[:], rbs[:])

        # ================ Phase 3: Wa1 + LN1 + transpose ================
        with tc.tile_pool(name="proj768b", bufs=2, space="PSUM") as proj768b, \
             tc.tile_pool(name="ps_tp", bufs=2, space="PSUM") as ps_tp, \
             tc.tile_pool(name="ln", bufs=4) as ln_pool:
            for t in range(QT):
                ps = proj768b.tile([128, H], mybir.dt.float32)
                for k in range(KH):
                    nc.tensor.matmul(
                        ps[:, 0:512], lhsT=CTX_sb[:, k, ts(t, 128)], rhs=WA1[:, k, 0:512],
                        start=(k == 0), stop=(k == KH - 1))
                for k in range(KH):
                    nc.tensor.matmul(
                        ps[:, 512:768], lhsT=CTX_sb[:, k, ts(t, 128)], rhs=WA1[:, k, 512:768],
                        start=(k == 0), stop=(k == KH - 1))
                _layer_norm_res(tc, ln_pool, ps, AN1[:, t, :], XQ[:, t, :], EPS_T)
                nc.vector.tensor_copy(AN1B[:, t, :], AN1[:, t, :])
            for m in range(KH):
                tp = ps_tp.tile([128, QC], bf16)
                for t in range(QT):
                    nc.tensor.transpose(tp[:, ts(t, 128)], AN1B[:, t, ts(m, 128)], IDN[:])
                nc.vector.tensor_copy(AN1T[:, m, :], tp[:])

        # ================ Phase 4: FFN ==================================
        with tc.tile_pool(name="w1s", bufs=4) as w1_pool, \
             tc.tile_pool(name="w2s", bufs=4) as w2_pool, \
             tc.tile_pool(name="g", bufs=3) as g_pool, \
             tc.tile_pool(name="ps_ff", bufs=1, space="PSUM") as ps_ff, \
             tc.tile_pool(name="ps_h1", bufs=2, space="PSUM") as ps_h1:
            ffps = []
            for m in range(KH):
                ffm = ps_ff.tile([128, QC], mybir.dt.float32, tag=f"ff{m}")
                ffps.append(ffm)
            for f in range(KF):
                w1t = w1_pool.tile([128, KH, 128], bf16)
                nc.sync.dma_start(out=w1t[:], in_=d_w1[f])
                h1 = ps_h1.tile([128, QC], mybir.dt.float32)
                for k in range(KH):
                    nc.tensor.matmul(
                        h1[:], lhsT=w1t[:, k, :], rhs=AN1T[:, k, :],
                        start=(k == 0), stop=(k == KH - 1))
                g = g_pool.tile([128, QC], bf16)
                nc.scalar.activation(g[:], h1[:], getattr(AF, act))
                w2t = w2_pool.tile([128, H], bf16)
                nc.sync.dma_start(out=w2t[:], in_=d_w2[f])
                for m in range(KH):
                    nc.tensor.matmul(
                        ffps[m][:], lhsT=w2t[:, ts(m, 128)], rhs=g[:],
                        start=(f == 0), stop=(f == KF - 1))
            for m in range(KH):
                nc.vector.tensor_copy(FFT_sb[:, m, :], ffps[m][:])

        # ================ Phase 5: Wa2 + LN2 + out ======================
        with tc.tile_pool(name="proj768c", bufs=2, space="PSUM") as proj768c, \
             tc.tile_pool(name="lnb", bufs=4) as ln_poolb, \
             tc.tile_pool(name="outp", bufs=2) as out_pool:
            for t in range(QT):
                ps = proj768c.tile([128, H], mybir.dt.float32)
                for k in range(KH):
                    nc.tensor.matmul(
                        ps[:, 0:512], lhsT=FFT_sb[:, k, ts(t, 128)], rhs=WA2[:, k, 0:512],
                        start=(k == 0), stop=(k == KH - 1))
                for k in range(KH):
                    nc.tensor.matmul(
                        ps[:, 512:768], lhsT=FFT_sb[:, k, ts(t, 128)], rhs=WA2[:, k, 512:768],
                        start=(k == 0), stop=(k == KH - 1))
                ot = out_pool.tile([128, H], mybir.dt.float32)
                _layer_norm_res(tc, ln_poolb, ps, ot[:], AN1[:, t, :], EPS_T)
                nc.sync.dma_start(out=d_out[:, t, :], in_=ot[:])

    nc.compile()
    return nc


def _layer_norm_res(tc, pool, ps, out_ap, res_ap, eps_t):
    """out = (ps - mean(ps)) / sqrt(var(ps)+eps) + res   (per-row over 768)."""
    import concourse.mybir as mybir
    nc = tc.nc
    AF = mybir.ActivationFunctionType
    stats = pool.tile([128, 3, 6], mybir.dt.float32, tag="stats")
    for sg in range(3):
        nc.vector.bn_stats(stats[:, sg, :], ps[:, sg * 256:(sg + 1) * 256])
    mv = pool.tile([128, 2], mybir.dt.float32, tag="mv")
    nc.vector.bn_aggr(mv[:], stats[:])
    rstd = pool.tile([128, 1], mybir.dt.float32, tag="rstd")
    nc.scalar.activation(rstd[:], mv[:, 1:2], AF.Sqrt, bias=eps_t[:])
    nc.vector.reciprocal(rstd[:], rstd[:])
    tmp = pool.tile([128, 768], mybir.dt.float32, tag="tmp")
    nc.vector.tensor_scalar(
        out=tmp[:], in0=ps[:], scalar1=mv[:, 0:1], scalar2=rstd[:],
        op0=mybir.AluOpType.subtract, op1=mybir.AluOpType.mult)
    nc.vector.tensor_add(out_ap, tmp[:], res_ap)


def _numpy_fallback(x, Wq, bq, Wk, bk, Wv, bv, Wa1, ba1, g1, be1,
                    W1, b1, W2, b2, Wa2, ba2, g2, be2):
    from scipy.special import erf

    def ln(v, g, b):
        mu = v.mean(-1, keepdims=True)
        var = ((v - mu) ** 2).mean(-1, keepdims=True)
        return (v - mu) / np.sqrt(var + EPS) * g + b

    out = np.zeros_like(x)
    for bi in range(x.shape[0]):
        xb = x[bi]
        q = (xb @ Wq + bq).reshape(S, NH, DH)
        k = (xb @ Wk + bk).reshape(S, NH, DH)
        v = (xb @ Wv + bv).reshape(S, NH, DH)
        ctx = np.zeros((S, NH, DH), np.float32)
        for h in range(NH):
            s = (q[:, h, :] @ k[:, h, :].T) / np.sqrt(np.float32(DH))
            s = s - s.max(-1, keepdims=True)
            e = np.exp(s)
            ctx[:, h, :] = (e / e.sum(-1, keepdims=True)) @ v[:, h, :]
        an1 = ln(ctx.reshape(S, H) @ Wa1 + ba1, g1, be1) + xb
        hh = an1 @ W1 + b1
        gg = hh * 0.5 * (1.0 + erf(hh / np.sqrt(2.0)))
        ff = gg @ W2 + b2
        out[bi] = ln(ff @ Wa2 + ba2, g2, be2) + an1
    return out


def _prep_inputs(inputs):
    """Host-side shard + cast.  Returns list of 8 in_maps."""
    x = np.asarray(inputs["x"], np.float32)

    def wtile(w):
        # [768, O] -> [128, 6, O] partition-major bf16
        return np.ascontiguousarray(
            w.reshape(KH, 128, -1).transpose(1, 0, 2)).astype(BF)

    wq = wtile(np.asarray(inputs["Wq"], np.float32) / np.sqrt(np.float32(DH)))
    wk = wtile(np.asarray(inputs["Wk"], np.float32))
    wv = wtile(np.asarray(inputs["Wv"], np.float32))
    wa1 = wtile(np.asarray(inputs["Wa1"], np.float32))
    wa2 = wtile(np.asarray(inputs["Wa2"], np.float32))
    w1 = np.ascontiguousarray(
        np.asarray(inputs["W1"], np.float32)
        .reshape(KH, 128, KF, 128).transpose(2, 1, 0, 3)).astype(BF)
    w2 = np.ascontiguousarray(
        np.asarray(inputs["W2"], np.float32).reshape(KF, 128, H)).astype(BF)

    in_maps = []
    for c in range(8):
        b, qi = divmod(c, 4)
        xb = x[b]                                        # [S, H]
        xT = np.ascontiguousarray(
            xb.T.reshape(KH, 128, S).transpose(1, 0, 2)).astype(BF)
        xqT = np.ascontiguousarray(xT[:, :, qi * QC:(qi + 1) * QC])
        xq = np.ascontiguousarray(
            xb[qi * QC:(qi + 1) * QC].reshape(QT, 128, H).transpose(1, 0, 2))
        in_maps.append(dict(
            xT=xT, xqT=xqT, xq=xq, wq=wq, wk=wk, wv=wv, wa1=wa1, wa2=wa2,
            w1=w1, w2=w2))
    return in_maps


def kernel(**inputs):
    # Generic fallback: the device fast-path assumes zero biases and unit
    # layernorm gains (true for this model's weights).
    zero_keys = ["bq", "bk", "bv", "ba1", "be1", "b1", "b2", "ba2", "be2"]
    if any(np.any(np.asarray(inputs[k]) != 0) for k in zero_keys) or \
       np.any(np.asarray(inputs["g1"]) != 1) or np.any(np.asarray(inputs["g2"]) != 1):
        return _numpy_fallback(
            **{k: np.asarray(v, np.float32) for k, v in inputs.items()})

    from concourse.bass_utils import run_bass_kernel_spmd

    if "nc" not in _CACHE:
        _CACHE["nc"] = _build_module()
    nc = _CACHE["nc"]

    in_maps = _prep_inputs(inputs)
    res = run_bass_kernel_spmd(nc, in_maps, core_ids=list(range(8)))
    out = np.zeros((B, S, H), np.float32)
    for c in range(8):
        b, qi = divmod(c, 4)
        o = res.results[c]["out"]                        # [128, QT, H]
        out[b, qi * QC:(qi + 1) * QC] = o.transpose(1, 0, 2).reshape(QC, H)
    return out


# revision 31
# speedup vs baseline: 1.0674x; 1.0674x over previous
"""BERT encoder layer (B=2, S=2048, H=768, NH=12, F=3072) on 8 TRN2 NeuronCores.

Sharding: pure data-parallel over (batch, query-chunk): core c handles batch
c//4, query rows (c%4)*512 .. +512.  Each core redundantly projects K/V for the
full 2048-token sequence of its batch (no collectives needed).

Numerics: bf16 matmul inputs with f32 PSUM accumulation; softmax without
max-subtraction (scores for this model are in [-3, 3]); normalization by the
softmax denominator is applied to the small ctx^T tile instead of the S^2
attention matrix.  LayerNorm stats in f32.

Layout strategy: "transposed activations" — projections produce K^T/Q^T
(feature-major) and V (token-major) so that attention scores^T, exp, ctx^T and
all later matmuls need no transposes except one 768x512 transpose of an1.
Attention head pairs are packed into the 128-wide PE array via tile_position
(Dh=64).
"""

import sys

for _p in ("/opt/trn_rl_repo",):
    if _p not in sys.path:
        sys.path.insert(0, _p)

import numpy as np
import ml_dtypes

H = 768
F = 3072
NH = 12
DH = 64
S = 2048
B = 2
QC = 512          # query rows per core
KH = H // 128     # 6 contraction chunks for H
KF = F // 128     # 24 contraction chunks for F
NT = S // 128     # 16 token tiles per sequence
QT = QC // 128    # 4 token tiles per core's query chunk
NPAIR = NH // 2   # 6 head pairs
EPS = 1e-5

BF = ml_dtypes.bfloat16

_CACHE = {}


def _build_module(act="Gelu"):
    """act: activation name for the FFN ("Gelu" for real runs; CoreSim lacks
    Gelu so dev-sim uses "Tanh")."""
    import concourse.bass as bass
    import concourse.tile as tile
    from concourse import bacc, mybir
    from concourse.masks import make_identity
    from contextlib import ExitStack

    bf16 = mybir.dt.bfloat16
    f32 = mybir.dt.float32
    ts = bass.ts
    AF = mybir.ActivationFunctionType

    nc = bacc.Bacc("TRN2", target_bir_lowering=False, debug=False)

    d_xT = nc.dram_tensor("xT", [128, KH, S], bf16, kind="ExternalInput").ap()
    d_xqT = nc.dram_tensor("xqT", [128, KH, QC], bf16, kind="ExternalInput").ap()
    d_xq = nc.dram_tensor("xq", [128, QT, H], f32, kind="ExternalInput").ap()
    d_wq = nc.dram_tensor("wq", [128, KH, H], bf16, kind="ExternalInput").ap()
    d_wk = nc.dram_tensor("wk", [128, KH, H], bf16, kind="ExternalInput").ap()
    d_wv = nc.dram_tensor("wv", [128, KH, H], bf16, kind="ExternalInput").ap()
    d_wa1 = nc.dram_tensor("wa1", [128, KH, H], bf16, kind="ExternalInput").ap()
    d_wa2 = nc.dram_tensor("wa2", [128, KH, H], bf16, kind="ExternalInput").ap()
    d_w1 = nc.dram_tensor("w1", [KF, 128, KH, 128], bf16, kind="ExternalInput").ap()
    d_w2 = nc.dram_tensor("w2", [KF, 128, H], bf16, kind="ExternalInput").ap()
    d_out = nc.dram_tensor("out", [128, QT, H], f32, kind="ExternalOutput").ap()
    d_zs = nc.dram_tensor("zscratch", [NPAIR, 2, QC], f32).ap()

    with tile.TileContext(nc) as tc, ExitStack() as ctx:
        singles = ctx.enter_context(tc.tile_pool(name="singles", bufs=1))

        # ---- resident SBUF tensors -------------------------------------
        XT = singles.tile([128, KH, S], bf16)
        nc.sync.dma_start(out=XT[:], in_=d_xT[:])
        WQ = singles.tile([128, KH, H], bf16)
        nc.sync.dma_start(out=WQ[:], in_=d_wq[:])
        WK = singles.tile([128, KH, H], bf16)
        nc.sync.dma_start(out=WK[:], in_=d_wk[:])
        WV = singles.tile([128, KH, H], bf16)
        nc.sync.dma_start(out=WV[:], in_=d_wv[:])
        WA1 = singles.tile([128, KH, H], bf16)
        nc.sync.dma_start(out=WA1[:], in_=d_wa1[:])
        WA2 = singles.tile([128, KH, H], bf16)
        nc.sync.dma_start(out=WA2[:], in_=d_wa2[:])
        XQ = singles.tile([128, QT, H], f32)
        nc.sync.dma_start(out=XQ[:], in_=d_xq[:])
        XQT = singles.tile([128, KH, QC], bf16)
        nc.sync.dma_start(out=XQT[:], in_=d_xqT[:])

        IDN = singles.tile([128, 128], bf16)
        make_identity(nc, IDN[:])
        EPS_T = singles.tile([128, 1], f32)
        nc.vector.memset(EPS_T[:], EPS)

        # V token-major, augmented with a ones column per head: lhsT
        # [V_h | 1] makes the ctx matmul also produce the softmax
        # denominator in output row 64.
        V_sb = singles.tile([128, NT, NH, DH + 1], bf16)
        nc.vector.memset(V_sb[:, :, :, DH:DH + 1], 1.0)
        QT_sb = singles.tile([128, KH, QC], bf16)    # Q^T feature-major
        CTX_sb = singles.tile([128, KH, QC], bf16)   # ctx^T feature-major
        AN1 = singles.tile([128, QT, H], f32)        # an1 token-major f32
        AN1T = singles.tile([128, KH, QC], bf16)     # an1^T
        FFT_sb = singles.tile([128, KH, QC], bf16)   # ff^T

        # ================ Phase 1: V (all heads) + Q^T ==================
        with tc.tile_pool(name="proj768", bufs=2, space="PSUM") as proj768:
            for m in range(NT):
                ps = proj768.tile([128, H], mybir.dt.float32)
                for k in range(KH):
                    nc.tensor.matmul(
                        ps[:, 0:512], lhsT=XT[:, k, ts(m, 128)], rhs=WV[:, k, 0:512],
                        start=(k == 0), stop=(k == KH - 1))
                for k in range(KH):
                    nc.tensor.matmul(
                        ps[:, 512:768], lhsT=XT[:, k, ts(m, 128)], rhs=WV[:, k, 512:768],
                        start=(k == 0), stop=(k == KH - 1))
                nc.vector.tensor_copy(
                    V_sb[:, m, :, 0:DH],
                    ps[:].rearrange("p (h d) -> p h d", h=NH))
            for mk in range(KH):
                ps = proj768.tile([128, QC], mybir.dt.float32, tag="qt")
                for k in range(KH):
                    nc.tensor.matmul(
                        ps[:], lhsT=WQ[:, k, ts(mk, 128)],
                        rhs=XQT[:, k, :],
                        start=(k == 0), stop=(k == KH - 1))
                nc.scalar.copy(QT_sb[:, mk, :], ps[:])

        # ================ Phase 2: attention by head pairs ==============
        # PE stream per pair: K^T projection (next pair's projection is
        # emitted under the tail of this pair's exps), row-split score
        # matmuls for the two heads, ctx matmuls with the denominator row.
        # Softmax normalization runs entirely off the PE queue
        # (DVE reciprocal + DMA broadcast + DVE multiply).
        with tc.tile_pool(name="kt", bufs=2) as kt_pool, \
             tc.tile_pool(name="exp", bufs=4) as exp_pool, \
             tc.tile_pool(name="rz", bufs=2) as rz_pool, \
             tc.tile_pool(name="rbs", bufs=4) as rbs_pool, \
             tc.tile_pool(name="ctmp", bufs=2) as ctmp_pool, \
             tc.tile_pool(name="ps_kt", bufs=1, space="PSUM") as ps_kt, \
             tc.tile_pool(name="ps_sc", bufs=2, space="PSUM") as ps_sc, \
             tc.tile_pool(name="ps_ctx", bufs=2, space="PSUM") as ps_ctx:

            def emit_kt(p):
                KT = kt_pool.tile([128, NT, 128], bf16, tag="kt", name=f"kt{p}")
                for n in range(4):
                    ps = ps_kt.tile([128, 512], mybir.dt.float32, tag="pskt",
                                    name=f"pskt{p}_{n}")
                    for k in range(KH):
                        nc.tensor.matmul(
                            ps[:], lhsT=WK[:, k, ts(p, 128)], rhs=XT[:, k, ts(n, 512)],
                            start=(k == 0), stop=(k == KH - 1))
                    nc.vector.tensor_copy(
                        KT[:, 4 * n:4 * (n + 1), :].rearrange("p a b -> p (a b)"),
                        ps[:])
                return KT

            KTs = {0: emit_kt(0)}
            for p in range(NPAIR):
                hA, hB = 2 * p, 2 * p + 1
                KT = KTs.pop(p)
                ctxA = ps_ctx.tile([DH + 1, QC], mybir.dt.float32, tag="ctx",
                                   name=f"cA{p}")
                ctxB = ps_ctx.tile([DH + 1, QC], mybir.dt.float32, tag="ctx",
                                   name=f"cB{p}")

                def emit_scores(kc):
                    sA = ps_sc.tile([128, 512], mybir.dt.float32, tag="sA",
                                    name=f"sA{p}_{kc}")
                    sB = ps_sc.tile([128, 512], mybir.dt.float32, tag="sB",
                                    name=f"sB{p}_{kc}")
                    nc.tensor.matmul(
                        sA[:], lhsT=KT[0:64, kc, :], rhs=QT_sb[0:64, p, :],
                        start=True, stop=True, tile_position=(0, 0))
                    nc.tensor.matmul(
                        sB[:], lhsT=KT[64:128, kc, :], rhs=QT_sb[64:128, p, :],
                        start=True, stop=True, tile_position=(64, 0))
                    return sA, sB

                pend = emit_scores(0)
                for kc in range(NT):
                    sA, sB = pend
                    if kc < NT - 1:
                        pend = emit_scores(kc + 1)
                    elif p + 1 < NPAIR:
                        KTs[p + 1] = emit_kt(p + 1)
                    eA = exp_pool.tile([128, 512], bf16, tag="e", name=f"eA{p}_{kc}")
                    eB = exp_pool.tile([128, 512], bf16, tag="e", name=f"eB{p}_{kc}")
                    nc.scalar.activation(eA[:], sA[:], AF.Exp)
                    nc.scalar.activation(eB[:], sB[:], AF.Exp)
                    nc.tensor.matmul(
                        ctxA[:], lhsT=V_sb[:, kc, hA, :], rhs=eA[:],
                        start=(kc == 0), stop=(kc == NT - 1))
                    nc.tensor.matmul(
                        ctxB[:], lhsT=V_sb[:, kc, hB, :], rhs=eB[:],
                        start=(kc == 0), stop=(kc == NT - 1))
                # softmax normalization: rows 0..63 are sum(exp*s .v),
                # row 64 is Z = sum(exp); scale rows by 1/Z.  All off-PE.
                rz = rz_pool.tile([DH + 1, 2, QC], mybir.dt.float32, tag="rz",
                                  name=f"rz{p}")
                nc.vector.reciprocal(rz[DH:DH + 1, 0, :], ctxA[DH:DH + 1, :])
                nc.vector.reciprocal(rz[DH:DH + 1, 1, :], ctxB[DH:DH + 1, :])
                rbsA = rbs_pool.tile([DH, QC], mybir.dt.float32, tag="rb", name=f"rbA{p}")
                rbsB = rbs_pool.tile([DH, QC], mybir.dt.float32, tag="rb", name=f"rbB{p}")
                nc.sync.dma_start(out=d_zs[p:p + 1], in_=rz[DH:DH + 1, :, :])

                def bcast_dram(src):
                    return bass.AP(
                        tensor=src.tensor, offset=src.offset,
                        ap=[[0, DH]] + list(src.ap))
                nc.sync.dma_start(out=rbsA[:], in_=bcast_dram(d_zs[p, 0, :]))
                nc.sync.dma_start(out=rbsB[:], in_=bcast_dram(d_zs[p, 1, :]))
                nc.vector.tensor_mul(CTX_sb[0:DH, p, :], ctxA[0:DH, :], rbsA[:])
                ctmp = ctmp_pool.tile([DH, QC], bf16, tag="ctmp", name=f"ctmp{p}")
                nc.vector.tensor_mul(ctmp[:], ctxB[0:DH, :], rbsB[:])
                nc.sync.dma_start(out=CTX_sb[DH:128, p, :], in_=ctmp[:])

        # ================ Phase 3: Wa1 + LN1 + transpose ================
        with tc.tile_pool(name="proj768b", bufs=2, space="PSUM") as proj768b, \
             tc.tile_pool(name="ps_tp", bufs=2, space="PSUM") as ps_tp, \
             tc.tile_pool(name="an1b", bufs=2) as an1b_pool, \
             tc.tile_pool(name="ln", bufs=4) as ln_pool:
            for t in range(QT):
                ps = proj768b.tile([128, H], mybir.dt.float32)
                for k in range(KH):
                    nc.tensor.matmul(
                        ps[:, 0:512], lhsT=CTX_sb[:, k, ts(t, 128)], rhs=WA1[:, k, 0:512],
                        start=(k == 0), stop=(k == KH - 1))
                for k in range(KH):
                    nc.tensor.matmul(
                        ps[:, 512:768], lhsT=CTX_sb[:, k, ts(t, 128)], rhs=WA1[:, k, 512:768],
                        start=(k == 0), stop=(k == KH - 1))
                _layer_norm_res(tc, ln_pool, ps, AN1[:, t, :], XQ[:, t, :], EPS_T)
                an1b = an1b_pool.tile([128, H], bf16, tag="an1b", name=f"an1b{t}")
                nc.vector.tensor_copy(an1b[:], AN1[:, t, :])
                tp = ps_tp.tile([128, KH, 128], bf16, tag="tp", name=f"tp{t}")
                for m in range(KH):
                    nc.tensor.transpose(tp[:, m, :], an1b[:, ts(m, 128)], IDN[:])
                for m in range(KH):
                    nc.vector.tensor_copy(AN1T[:, m, ts(t, 128)], tp[:, m, :])

        # ================ Phase 4: FFN ==================================
        with tc.tile_pool(name="w1s", bufs=4) as w1_pool, \
             tc.tile_pool(name="w2s", bufs=4) as w2_pool, \
             tc.tile_pool(name="g", bufs=3) as g_pool, \
             tc.tile_pool(name="ps_ff", bufs=1, space="PSUM") as ps_ff, \
             tc.tile_pool(name="ps_h1", bufs=2, space="PSUM") as ps_h1:
            ffps = []
            for m in range(KH):
                ffm = ps_ff.tile([128, QC], mybir.dt.float32, tag=f"ff{m}")
                ffps.append(ffm)
            for f in range(KF):
                w1t = w1_pool.tile([128, KH, 128], bf16)
                nc.sync.dma_start(out=w1t[:], in_=d_w1[f])
                h1 = ps_h1.tile([128, QC], mybir.dt.float32)
                for k in range(KH):
                    nc.tensor.matmul(
                        h1[:], lhsT=w1t[:, k, :], rhs=AN1T[:, k, :],
                        start=(k == 0), stop=(k == KH - 1))
                g = g_pool.tile([128, QC], bf16)
                nc.scalar.activation(g[:], h1[:], getattr(AF, act))
                w2t = w2_pool.tile([128, H], bf16)
                nc.sync.dma_start(out=w2t[:], in_=d_w2[f])
                for m in range(KH):
                    nc.tensor.matmul(
                        ffps[m][:], lhsT=w2t[:, ts(m, 128)], rhs=g[:],
                        start=(f == 0), stop=(f == KF - 1))
            for m in range(KH):
                nc.vector.tensor_copy(FFT_sb[:, m, :], ffps[m][:])

        # ================ Phase 5: Wa2 + LN2 + out ======================
        with tc.tile_pool(name="proj768c", bufs=2, space="PSUM") as proj768c, \
             tc.tile_pool(name="lnb", bufs=4) as ln_poolb, \
             tc.tile_pool(name="outp", bufs=2) as out_pool:
            for t in range(QT):
                ps = proj768c.tile([128, H], mybir.dt.float32)
                for k in range(KH):
                    nc.tensor.matmul(
                        ps[:, 0:512], lhsT=FFT_sb[:, k, ts(t, 128)], rhs=WA2[:, k, 0:512],
                        start=(k == 0), stop=(k == KH - 1))
                for k in range(KH):
                    nc.tensor.matmul(
                        ps[:, 512:768], lhsT=FFT_sb[:, k, ts(t, 128)], rhs=WA2[:, k, 512:768],
                        start=(k == 0), stop=(k == KH - 1))
                ot = out_pool.tile([128, H], mybir.dt.float32)
                _layer_norm_res(tc, ln_poolb, ps, ot[:], AN1[:, t, :], EPS_T)
                nc.sync.dma_start(out=d_out[:, t, :], in_=ot[:])

    nc.compile()
    return nc


def _layer_norm_res(tc, pool, ps, out_ap, res_ap, eps_t):
    """out = (ps - mean(ps)) / sqrt(var(ps)+eps) + res   (per-row over 768)."""
    import concourse.mybir as mybir
    nc = tc.nc
    AF = mybir.ActivationFunctionType
    stats = pool.tile([128, 3, 6], mybir.dt.float32, tag="stats")
    for sg in range(3):
        nc.vector.bn_stats(stats[:, sg, :], ps[:, sg * 256:(sg + 1) * 256])
    mv = pool.tile([128, 2], mybir.dt.float32, tag="mv")
    nc.vector.bn_aggr(mv[:], stats[:])
    # rstd = exp(-0.5 * ln(var + eps)) — Ln and Exp share an ACT table set
    # with the attention exp, avoiding table reloads.
    rstd = pool.tile([128, 1], mybir.dt.float32, tag="rstd")
    nc.scalar.activation(rstd[:], mv[:, 1:2], AF.Ln, bias=eps_t[:])
    nc.scalar.activation(rstd[:], rstd[:], AF.Exp, scale=-0.5)
    tmp = pool.tile([128, 768], mybir.dt.float32, tag="tmp")
    nc.vector.tensor_scalar(
        out=tmp[:], in0=ps[:], scalar1=mv[:, 0:1], scalar2=rstd[:],
        op0=mybir.AluOpType.subtract, op1=mybir.AluOpType.mult)
    nc.vector.tensor_add(out_ap, tmp[:], res_ap)


def _numpy_fallback(x, Wq, bq, Wk, bk, Wv, bv, Wa1, ba1, g1, be1,
                    W1, b1, W2, b2, Wa2, ba2, g2, be2):
    from scipy.special import erf

    def ln(v, g, b):
        mu = v.mean(-1, keepdims=True)
        var = ((v - mu) ** 2).mean(-1, keepdims=True)
        return (v - mu) / np.sqrt(var + EPS) * g + b

    out = np.zeros_like(x)
    for bi in range(x.shape[0]):
        xb = x[bi]
        q = (xb @ Wq + bq).reshape(S, NH, DH)
        k = (xb @ Wk + bk).reshape(S, NH, DH)
        v = (xb @ Wv + bv).reshape(S, NH, DH)
        ctx = np.zeros((S, NH, DH), np.float32)
        for h in range(NH):
            s = (q[:, h, :] @ k[:, h, :].T) / np.sqrt(np.float32(DH))
            s = s - s.max(-1, keepdims=True)
            e = np.exp(s)
            ctx[:, h, :] = (e / e.sum(-1, keepdims=True)) @ v[:, h, :]
        an1 = ln(ctx.reshape(S, H) @ Wa1 + ba1, g1, be1) + xb
        hh = an1 @ W1 + b1
        gg = hh * 0.5 * (1.0 + erf(hh / np.sqrt(2.0)))
        ff = gg @ W2 + b2
        out[bi] = ln(ff @ Wa2 + ba2, g2, be2) + an1
    return out


def _prep_inputs(inputs):
    """Host-side shard + cast.  Returns list of 8 in_maps."""
    x = np.asarray(inputs["x"], np.float32)

    def wtile(w):
        # [768, O] -> [128, 6, O] partition-major bf16
        return np.ascontiguousarray(
            w.reshape(KH, 128, -1).transpose(1, 0, 2)).astype(BF)

    wq = wtile(np.asarray(inputs["Wq"], np.float32) / np.sqrt(np.float32(DH)))
    wk = wtile(np.asarray(inputs["Wk"], np.float32))
    wv = wtile(np.asarray(inputs["Wv"], np.float32))
    wa1 = wtile(np.asarray(inputs["Wa1"], np.float32))
    wa2 = wtile(np.asarray(inputs["Wa2"], np.float32))
    w1 = np.ascontiguousarray(
        np.asarray(inputs["W1"], np.float32)
        .reshape(KH, 128, KF, 128).transpose(2, 1, 0, 3)).astype(BF)
    w2 = np.ascontiguousarray(
        np.asarray(inputs["W2"], np.float32).reshape(KF, 128, H)).astype(BF)

    in_maps = []
    for c in range(8):
        b, qi = divmod(c, 4)
        xb = x[b]                                        # [S, H]
        xT = np.ascontiguousarray(
            xb.T.reshape(KH, 128, S).transpose(1, 0, 2)).astype(BF)
        xqT = np.ascontiguousarray(xT[:, :, qi * QC:(qi + 1) * QC])
        xq = np.ascontiguousarray(
            xb[qi * QC:(qi + 1) * QC].reshape(QT, 128, H).transpose(1, 0, 2))
        in_maps.append(dict(
            xT=xT, xqT=xqT, xq=xq, wq=wq, wk=wk, wv=wv, wa1=wa1, wa2=wa2,
            w1=w1, w2=w2))
    return in_maps


def kernel(**inputs):
    # Generic fallback: the device fast-path assumes zero biases and unit
    # layernorm gains (true for this model's weights).
    zero_keys = ["bq", "bk", "bv", "ba1", "be1", "b1", "b2", "ba2", "be2"]
    if any(np.any(np.asarray(inputs[k]) != 0) for k in zero_keys) or \
       np.any(np.asarray(inputs["g1"]) != 1) or np.any(np.asarray(inputs["g2"]) != 1):
        return _numpy_fallback(
            **{k: np.asarray(v, np.float32) for k, v in inputs.items()})

    from concourse.bass_utils import run_bass_kernel_spmd

    if "nc" not in _CACHE:
        _CACHE["nc"] = _build_module()
    nc = _CACHE["nc"]

    in_maps = _prep_inputs(inputs)
    res = run_bass_kernel_spmd(nc, in_maps, core_ids=list(range(8)))
    out = np.zeros((B, S, H), np.float32)
    for c in range(8):
        b, qi = divmod(c, 4)
        o = res.results[c]["out"]                        # [128, QT, H]
        out[b, qi * QC:(qi + 1) * QC] = o.transpose(1, 0, 2).reshape(QC, H)
    return out


# revision 36
# speedup vs baseline: 1.2059x; 1.1297x over previous
"""BERT encoder layer (B=2, S=2048, H=768, NH=12, F=3072) on 8 TRN2 NeuronCores.

Sharding: pure data-parallel over (batch, query-chunk): core c handles batch
c//4, query rows (c%4)*512 .. +512.  Each core redundantly projects K/V for the
full 2048-token sequence of its batch (no collectives needed).

Numerics: bf16 matmul inputs with f32 PSUM accumulation; softmax without
max-subtraction (scores for this model are in [-3, 3]); normalization by the
softmax denominator is applied to the small ctx^T tile instead of the S^2
attention matrix.  LayerNorm stats in f32.

Layout strategy: "transposed activations" — projections produce K^T/Q^T
(feature-major) and V (token-major) so that attention scores^T, exp, ctx^T and
all later matmuls need no transposes except one 768x512 transpose of an1.
Attention head pairs are packed into the 128-wide PE array via tile_position
(Dh=64).
"""

import sys

for _p in ("/opt/trn_rl_repo",):
    if _p not in sys.path:
        sys.path.insert(0, _p)

import numpy as np
import ml_dtypes

H = 768
F = 3072
NH = 12
DH = 64
S = 2048
B = 2
QC = 512          # query rows per core
KH = H // 128     # 6 contraction chunks for H
KF = F // 128     # 24 contraction chunks for F
NT = S // 128     # 16 token tiles per sequence
QT = QC // 128    # 4 token tiles per core's query chunk
NPAIR = NH // 2   # 6 head pairs
EPS = 1e-5

BF = ml_dtypes.bfloat16

_CACHE = {}


def _build_module(act="Gelu"):
    """act: activation name for the FFN ("Gelu" for real runs; CoreSim lacks
    Gelu so dev-sim uses "Tanh")."""
    import concourse.bass as bass
    import concourse.tile as tile
    from concourse import bacc, mybir
    from concourse.masks import make_identity
    from contextlib import ExitStack

    bf16 = mybir.dt.bfloat16
    f32 = mybir.dt.float32
    ts = bass.ts
    AF = mybir.ActivationFunctionType

    nc = bacc.Bacc("TRN2", target_bir_lowering=False, debug=False)

    d_xT = nc.dram_tensor("xT", [128, KH, S], bf16, kind="ExternalInput").ap()
    d_xqT = nc.dram_tensor("xqT", [128, KH, QC], bf16, kind="ExternalInput").ap()
    d_xq = nc.dram_tensor("xq", [128, QT, H], f32, kind="ExternalInput").ap()
    d_wq = nc.dram_tensor("wq", [128, KH, H], bf16, kind="ExternalInput").ap()
    d_wk = nc.dram_tensor("wk", [128, KH, H], bf16, kind="ExternalInput").ap()
    d_wv = nc.dram_tensor("wv", [128, KH, H], bf16, kind="ExternalInput").ap()
    d_wa1 = nc.dram_tensor("wa1", [128, KH, H], bf16, kind="ExternalInput").ap()
    d_wa2 = nc.dram_tensor("wa2", [128, KH, H], bf16, kind="ExternalInput").ap()
    d_w1 = nc.dram_tensor("w1", [KF, 128, KH, 128], bf16, kind="ExternalInput").ap()
    d_w2 = nc.dram_tensor("w2", [KF, 128, H], bf16, kind="ExternalInput").ap()
    d_out = nc.dram_tensor("out", [128, QT, H], f32, kind="ExternalOutput").ap()
    d_zs = nc.dram_tensor("zscratch", [NPAIR, 2, QC], f32).ap()

    with tile.TileContext(nc) as tc, ExitStack() as ctx:
        singles = ctx.enter_context(tc.tile_pool(name="singles", bufs=1))

        # ---- resident SBUF tensors -------------------------------------
        # DMA order matters at startup: V projection needs XT+WV first.
        XT = singles.tile([128, KH, S], bf16)
        nc.sync.dma_start(out=XT[:], in_=d_xT[:])
        WV = singles.tile([128, KH, H], bf16)
        nc.sync.dma_start(out=WV[:], in_=d_wv[:])
        WQ = singles.tile([128, KH, H], bf16)
        nc.sync.dma_start(out=WQ[:], in_=d_wq[:])
        XQT = singles.tile([128, KH, QC], bf16)
        nc.sync.dma_start(out=XQT[:], in_=d_xqT[:])
        WK = singles.tile([128, KH, H], bf16)
        nc.sync.dma_start(out=WK[:], in_=d_wk[:])
        WA1 = singles.tile([128, KH, H], bf16)
        nc.sync.dma_start(out=WA1[:], in_=d_wa1[:])
        WA2 = singles.tile([128, KH, H], bf16)
        nc.sync.dma_start(out=WA2[:], in_=d_wa2[:])
        XQ = singles.tile([128, QT, H], f32)
        nc.sync.dma_start(out=XQ[:], in_=d_xq[:])

        IDN = singles.tile([128, 128], bf16)
        make_identity(nc, IDN[:])
        EPS_T = singles.tile([128, 1], f32)
        nc.vector.memset(EPS_T[:], EPS)

        # V token-major, augmented with a ones column per head: lhsT
        # [V_h | 1] makes the ctx matmul also produce the softmax
        # denominator in output row 64.
        V_sb = singles.tile([128, NT, NH, DH + 1], bf16)
        nc.vector.memset(V_sb[:, :, :, DH:DH + 1], 1.0)
        QT_sb = singles.tile([128, KH, QC], bf16)    # Q^T feature-major
        CTX_sb = singles.tile([128, KH, QC], bf16)   # ctx^T feature-major
        AN1 = singles.tile([128, QT, H], f32)        # an1 token-major f32
        AN1T = singles.tile([128, KH, QC], bf16)     # an1^T
        FFT_sb = singles.tile([128, KH, QC], bf16)   # ff^T

        # ================ Phase 1: V (all heads) + Q^T ==================
        with tc.tile_pool(name="proj768", bufs=2, space="PSUM") as proj768:
            for m in range(NT):
                ps = proj768.tile([128, H], mybir.dt.float32)
                for k in range(KH):
                    nc.tensor.matmul(
                        ps[:, 0:512], lhsT=XT[:, k, ts(m, 128)], rhs=WV[:, k, 0:512],
                        start=(k == 0), stop=(k == KH - 1))
                for k in range(KH):
                    nc.tensor.matmul(
                        ps[:, 512:768], lhsT=XT[:, k, ts(m, 128)], rhs=WV[:, k, 512:768],
                        start=(k == 0), stop=(k == KH - 1))
                nc.vector.tensor_copy(
                    V_sb[:, m, :, 0:DH],
                    ps[:].rearrange("p (h d) -> p h d", h=NH))
            for mk in range(KH):
                ps = proj768.tile([128, QC], mybir.dt.float32, tag="qt")
                for k in range(KH):
                    nc.tensor.matmul(
                        ps[:], lhsT=WQ[:, k, ts(mk, 128)],
                        rhs=XQT[:, k, :],
                        start=(k == 0), stop=(k == KH - 1))
                nc.scalar.copy(QT_sb[:, mk, :], ps[:])

        # ================ Phase 2: attention by head pairs ==============
        # PE stream per pair: K^T projection (next pair's projection is
        # emitted under the tail of this pair's exps), row-split score
        # matmuls for the two heads, ctx matmuls with the denominator row.
        # Softmax normalization runs entirely off the PE queue
        # (DVE reciprocal + DMA broadcast + DVE multiply).
        with tc.tile_pool(name="kt", bufs=2) as kt_pool, \
             tc.tile_pool(name="exp", bufs=4) as exp_pool, \
             tc.tile_pool(name="rz", bufs=1) as rz_pool, \
             tc.tile_pool(name="rbs", bufs=4) as rbs_pool, \
             tc.tile_pool(name="ctmp", bufs=2) as ctmp_pool, \
             tc.tile_pool(name="ps_kt", bufs=1, space="PSUM") as ps_kt, \
             tc.tile_pool(name="ps_sc", bufs=2, space="PSUM") as ps_sc, \
             tc.tile_pool(name="ps_ctx", bufs=3, space="PSUM") as ps_ctx:

            def emit_kt(p):
                KT = kt_pool.tile([128, NT, 128], bf16, tag="kt", name=f"kt{p}")
                for n in range(4):
                    ps = ps_kt.tile([128, 512], mybir.dt.float32, tag="pskt",
                                    name=f"pskt{p}_{n}")
                    for k in range(KH):
                        nc.tensor.matmul(
                            ps[:], lhsT=WK[:, k, ts(p, 128)], rhs=XT[:, k, ts(n, 512)],
                            start=(k == 0), stop=(k == KH - 1))
                    nc.vector.tensor_copy(
                        KT[:, 4 * n:4 * (n + 1), :].rearrange("p a b -> p (a b)"),
                        ps[:])
                return KT

            KTs = {0: emit_kt(0)}
            for p in range(NPAIR):
                hA, hB = 2 * p, 2 * p + 1
                KT = KTs.pop(p)
                ctxA = ps_ctx.tile([DH + 1, QC], mybir.dt.float32, tag="ctx",
                                   name=f"cA{p}")
                ctxB = ps_ctx.tile([DH + 1, QC], mybir.dt.float32, tag="ctx",
                                   name=f"cB{p}")

                def emit_scores(kc):
                    sA = ps_sc.tile([128, 512], mybir.dt.float32, tag="sA",
                                    name=f"sA{p}_{kc}")
                    sB = ps_sc.tile([128, 512], mybir.dt.float32, tag="sB",
                                    name=f"sB{p}_{kc}")
                    nc.tensor.matmul(
                        sA[:], lhsT=KT[0:64, kc, :], rhs=QT_sb[0:64, p, :],
                        start=True, stop=True, tile_position=(0, 0))
                    nc.tensor.matmul(
                        sB[:], lhsT=KT[64:128, kc, :], rhs=QT_sb[64:128, p, :],
                        start=True, stop=True, tile_position=(64, 0))
                    return sA, sB

                pend = emit_scores(0)
                for kc in range(NT):
                    sA, sB = pend
                    if kc < NT - 1:
                        pend = emit_scores(kc + 1)
                    elif p + 1 < NPAIR:
                        KTs[p + 1] = emit_kt(p + 1)
                    eA = exp_pool.tile([128, 512], bf16, tag="e", name=f"eA{p}_{kc}")
                    eB = exp_pool.tile([128, 512], bf16, tag="e", name=f"eB{p}_{kc}")
                    nc.scalar.activation(eA[:], sA[:], AF.Exp)
                    nc.scalar.activation(eB[:], sB[:], AF.Exp)
                    nc.tensor.matmul(
                        ctxA[:], lhsT=V_sb[:, kc, hA, :], rhs=eA[:],
                        start=(kc == 0), stop=(kc == NT - 1))
                    nc.tensor.matmul(
                        ctxB[:], lhsT=V_sb[:, kc, hB, :], rhs=eB[:],
                        start=(kc == 0), stop=(kc == NT - 1))
                # softmax normalization: rows 0..63 are sum(exp . v), row 64
                # is Z = sum(exp); scale rows by 1/Z.  Entirely off the PE
                # queue: copy ctx out of PSUM at once (frees the bank for the
                # next pair), 1/Z = exp(-ln Z) on ACT, partition-broadcast via
                # a DRAM round-trip, scale on DVE.
                cst = rz_pool.tile([DH + 1, 2, QC], mybir.dt.float32, tag="cst",
                                   name=f"cst{p}")
                nc.vector.tensor_copy(cst[:, 0, :], ctxA[:])
                nc.vector.tensor_copy(cst[:, 1, :], ctxB[:])
                rz = rz_pool.tile([DH + 1, 2, QC], mybir.dt.float32, tag="rz",
                                  name=f"rz{p}")
                nc.scalar.activation(rz[DH:DH + 1, :, :], cst[DH:DH + 1, :, :], AF.Ln)
                nc.scalar.activation(rz[DH:DH + 1, :, :], rz[DH:DH + 1, :, :],
                                     AF.Exp, scale=-1.0)
                rbsA = rbs_pool.tile([DH, QC], mybir.dt.float32, tag="rb", name=f"rbA{p}")
                rbsB = rbs_pool.tile([DH, QC], mybir.dt.float32, tag="rb", name=f"rbB{p}")
                nc.sync.dma_start(out=d_zs[p:p + 1], in_=rz[DH:DH + 1, :, :])

                def bcast_dram(src):
                    return bass.AP(
                        tensor=src.tensor, offset=src.offset,
                        ap=[[0, DH]] + list(src.ap))
                nc.sync.dma_start(out=rbsA[:], in_=bcast_dram(d_zs[p, 0, :]))
                nc.sync.dma_start(out=rbsB[:], in_=bcast_dram(d_zs[p, 1, :]))
                nc.vector.tensor_mul(CTX_sb[0:DH, p, :], cst[0:DH, 0, :], rbsA[:])
                ctmp = ctmp_pool.tile([DH, QC], bf16, tag="ctmp", name=f"ctmp{p}")
                nc.vector.tensor_mul(ctmp[:], cst[0:DH, 1, :], rbsB[:])
                nc.sync.dma_start(out=CTX_sb[DH:128, p, :], in_=ctmp[:])

        # ================ Phase 3: Wa1 + LN1 + transpose ================
        with tc.tile_pool(name="proj768b", bufs=2, space="PSUM") as proj768b, \
             tc.tile_pool(name="ps_tp", bufs=2, space="PSUM") as ps_tp, \
             tc.tile_pool(name="an1b", bufs=2) as an1b_pool, \
             tc.tile_pool(name="ln", bufs=4) as ln_pool:
            for t in range(QT):
                ps = proj768b.tile([128, H], mybir.dt.float32)
                for k in range(KH):
                    nc.tensor.matmul(
                        ps[:, 0:512], lhsT=CTX_sb[:, k, ts(t, 128)], rhs=WA1[:, k, 0:512],
                        start=(k == 0), stop=(k == KH - 1))
                for k in range(KH):
                    nc.tensor.matmul(
                        ps[:, 512:768], lhsT=CTX_sb[:, k, ts(t, 128)], rhs=WA1[:, k, 512:768],
                        start=(k == 0), stop=(k == KH - 1))
                _layer_norm_res(tc, ln_pool, ps, AN1[:, t, :], XQ[:, t, :], EPS_T)
                an1b = an1b_pool.tile([128, H], bf16, tag="an1b", name=f"an1b{t}")
                nc.vector.tensor_copy(an1b[:], AN1[:, t, :])
                tp = ps_tp.tile([128, KH, 128], bf16, tag="tp", name=f"tp{t}")
                for m in range(KH):
                    nc.tensor.transpose(tp[:, m, :], an1b[:, ts(m, 128)], IDN[:])
                for m in range(KH):
                    nc.vector.tensor_copy(AN1T[:, m, ts(t, 128)], tp[:, m, :])

        # ================ Phase 4: FFN ==================================
        with tc.tile_pool(name="w1s", bufs=6) as w1_pool, \
             tc.tile_pool(name="w2s", bufs=6) as w2_pool, \
             tc.tile_pool(name="g", bufs=3) as g_pool, \
             tc.tile_pool(name="ps_ff", bufs=1, space="PSUM") as ps_ff, \
             tc.tile_pool(name="ps_h1", bufs=2, space="PSUM") as ps_h1:
            ffps = []
            for m in range(KH):
                ffm = ps_ff.tile([128, QC], mybir.dt.float32, tag=f"ff{m}")
                ffps.append(ffm)

            def emit_h1(f):
                w1t = w1_pool.tile([128, KH, 128], bf16, tag="w1t", name=f"w1t{f}")
                nc.sync.dma_start(out=w1t[:], in_=d_w1[f])
                h1 = ps_h1.tile([128, QC], mybir.dt.float32, tag="h1", name=f"h1_{f}")
                for k in range(KH):
                    nc.tensor.matmul(
                        h1[:], lhsT=w1t[:, k, :], rhs=AN1T[:, k, :],
                        start=(k == 0), stop=(k == KH - 1))
                return h1

            # software pipeline: h1(f+1) matmuls are emitted before the
            # gelu(f)-dependent ff matmuls so PE never waits on ACT.
            pend_h1 = emit_h1(0)
            for f in range(KF):
                h1 = pend_h1
                g = g_pool.tile([128, QC], bf16, tag="g", name=f"g{f}")
                nc.scalar.activation(g[:], h1[:], getattr(AF, act))
                if f + 1 < KF:
                    pend_h1 = emit_h1(f + 1)
                w2t = w2_pool.tile([128, H], bf16, tag="w2t", name=f"w2t{f}")
                nc.sync.dma_start(out=w2t[:], in_=d_w2[f])
                for m in range(KH):
                    nc.tensor.matmul(
                        ffps[m][:], lhsT=w2t[:, ts(m, 128)], rhs=g[:],
                        start=(f == 0), stop=(f == KF - 1))
            for m in range(KH):
                nc.vector.tensor_copy(FFT_sb[:, m, :], ffps[m][:])

        # ================ Phase 5: Wa2 + LN2 + out ======================
        with tc.tile_pool(name="proj768c", bufs=2, space="PSUM") as proj768c, \
             tc.tile_pool(name="lnb", bufs=4) as ln_poolb, \
             tc.tile_pool(name="outp", bufs=2) as out_pool:
            for t in range(QT):
                ps = proj768c.tile([128, H], mybir.dt.float32)
                for k in range(KH):
                    nc.tensor.matmul(
                        ps[:, 0:512], lhsT=FFT_sb[:, k, ts(t, 128)], rhs=WA2[:, k, 0:512],
                        start=(k == 0), stop=(k == KH - 1))
                for k in range(KH):
                    nc.tensor.matmul(
                        ps[:, 512:768], lhsT=FFT_sb[:, k, ts(t, 128)], rhs=WA2[:, k, 512:768],
                        start=(k == 0), stop=(k == KH - 1))
                ot = out_pool.tile([128, H], mybir.dt.float32)
                _layer_norm_res(tc, ln_poolb, ps, ot[:], AN1[:, t, :], EPS_T)
                nc.sync.dma_start(out=d_out[:, t, :], in_=ot[:])

    nc.compile()
    return nc


def _layer_norm_res(tc, pool, ps, out_ap, res_ap, eps_t):
    """out = (ps - mean(ps)) / sqrt(var(ps)+eps) + res   (per-row over 768)."""
    import concourse.mybir as mybir
    nc = tc.nc
    AF = mybir.ActivationFunctionType
    stats = pool.tile([128, 3, 6], mybir.dt.float32, tag="stats")
    for sg in range(3):
        nc.vector.bn_stats(stats[:, sg, :], ps[:, sg * 256:(sg + 1) * 256])
    mv = pool.tile([128, 2], mybir.dt.float32, tag="mv")
    nc.vector.bn_aggr(mv[:], stats[:])
    # rstd = exp(-0.5 * ln(var + eps)) — Ln and Exp share an ACT table set
    # with the attention exp, avoiding table reloads.
    rstd = pool.tile([128, 1], mybir.dt.float32, tag="rstd")
    nc.scalar.activation(rstd[:], mv[:, 1:2], AF.Ln, bias=eps_t[:])
    nc.scalar.activation(rstd[:], rstd[:], AF.Exp, scale=-0.5)
    tmp = pool.tile([128, 768], mybir.dt.float32, tag="tmp")
    nc.vector.tensor_scalar(
        out=tmp[:], in0=ps[:], scalar1=mv[:, 0:1], scalar2=rstd[:],
        op0=mybir.AluOpType.subtract, op1=mybir.AluOpType.mult)
    nc.vector.tensor_add(out_ap, tmp[:], res_ap)


def _numpy_fallback(x, Wq, bq, Wk, bk, Wv, bv, Wa1, ba1, g1, be1,
                    W1, b1, W2, b2, Wa2, ba2, g2, be2):
    from scipy.special import erf

    def ln(v, g, b):
        mu = v.mean(-1, keepdims=True)
        var = ((v - mu) ** 2).mean(-1, keepdims=True)
        return (v - mu) / np.sqrt(var + EPS) * g + b

    out = np.zeros_like(x)
    for bi in range(x.shape[0]):
        xb = x[bi]
        q = (xb @ Wq + bq).reshape(S, NH, DH)
        k = (xb @ Wk + bk).reshape(S, NH, DH)
        v = (xb @ Wv + bv).reshape(S, NH, DH)
        ctx = np.zeros((S, NH, DH), np.float32)
        for h in range(NH):
            s = (q[:, h, :] @ k[:, h, :].T) / np.sqrt(np.float32(DH))
            s = s - s.max(-1, keepdims=True)
            e = np.exp(s)
            ctx[:, h, :] = (e / e.sum(-1, keepdims=True)) @ v[:, h, :]
        an1 = ln(ctx.reshape(S, H) @ Wa1 + ba1, g1, be1) + xb
        hh = an1 @ W1 + b1
        gg = hh * 0.5 * (1.0 + erf(hh / np.sqrt(2.0)))
        ff = gg @ W2 + b2
        out[bi] = ln(ff @ Wa2 + ba2, g2, be2) + an1
    return out


def _prep_inputs(inputs):
    """Host-side shard + cast.  Returns list of 8 in_maps."""
    x = np.asarray(inputs["x"], np.float32)

    def wtile(w):
        # [768, O] -> [128, 6, O] partition-major bf16
        return np.ascontiguousarray(
            w.reshape(KH, 128, -1).transpose(1, 0, 2)).astype(BF)

    wq = wtile(np.asarray(inputs["Wq"], np.float32) / np.sqrt(np.float32(DH)))
    wk = wtile(np.asarray(inputs["Wk"], np.float32))
    wv = wtile(np.asarray(inputs["Wv"], np.float32))
    wa1 = wtile(np.asarray(inputs["Wa1"], np.float32))
    wa2 = wtile(np.asarray(inputs["Wa2"], np.float32))
    w1 = np.ascontiguousarray(
        np.asarray(inputs["W1"], np.float32)
        .reshape(KH, 128, KF, 128).transpose(2, 1, 0, 3)).astype(BF)
    w2 = np.ascontiguousarray(
        np.asarray(inputs["W2"], np.float32).reshape(KF, 128, H)).astype(BF)

    in_maps = []
    for c in range(8):
        b, qi = divmod(c, 4)
        xb = x[b]                                        # [S, H]
        xT = np.ascontiguousarray(
            xb.T.reshape(KH, 128, S).transpose(1, 0, 2)).astype(BF)
        xqT = np.ascontiguousarray(xT[:, :, qi * QC:(qi + 1) * QC])
        xq = np.ascontiguousarray(
            xb[qi * QC:(qi + 1) * QC].reshape(QT, 128, H).transpose(1, 0, 2))
        in_maps.append(dict(
            xT=xT, xqT=xqT, xq=xq, wq=wq, wk=wk, wv=wv, wa1=wa1, wa2=wa2,
            w1=w1, w2=w2))
    return in_maps


def kernel(**inputs):
    # Generic fallback: the device fast-path assumes zero biases and unit
    # layernorm gains (true for this model's weights).
    zero_keys = ["bq", "bk", "bv", "ba1", "be1", "b1", "b2", "ba2", "be2"]
    if any(np.any(np.asarray(inputs[k]) != 0) for k in zero_keys) or \
       np.any(np.asarray(inputs["g1"]) != 1) or np.any(np.asarray(inputs["g2"]) != 1):
        return _numpy_fallback(
            **{k: np.asarray(v, np.float32) for k, v in inputs.items()})

    from concourse.bass_utils import run_bass_kernel_spmd

    if "nc" not in _CACHE:
        _CACHE["nc"] = _build_module()
    nc = _CACHE["nc"]

    in_maps = _prep_inputs(inputs)
    res = run_bass_kernel_spmd(nc, in_maps, core_ids=list(range(8)))
    out = np.zeros((B, S, H), np.float32)
    for c in range(8):
        b, qi = divmod(c, 4)
        o = res.results[c]["out"]                        # [128, QT, H]
        out[b, qi * QC:(qi + 1) * QC] = o.transpose(1, 0, 2).reshape(QC, H)
    return out
